# revision 25
# baseline (speedup 1.0000x reference)
"""Trainium2 Bass kernel: 3x3 conv (NCHW 32x256x56x56, 256->256ch, pad 1) with
a host-expanded synthesized weight, data-parallel over 8 NeuronCores.

Vertical 1D Winograd F(2,3): for each output row pair (y=2p, 2p+1) the three
vertical taps collapse into 4 Winograd products, cutting PE work 1.5x vs
direct implicit GEMM (12 row-matmuls per 2 rows -> 8).

  t0 = x[2p] - x[2p+2]      u0 = g0            out[2p]   = m0+m1+m2 + b
  t1 = x[2p+1] + x[2p+2]    u1 = (g0+g1+g2)/2  out[2p+1] = m1-m2-m3 + b
  t2 = x[2p+2] - x[2p+1]    u2 = (g0-g1+g2)/2
  t3 = x[2p+1] - x[2p+3]    u3 = g2            m_i = sum_{ci,dx} t_i * u_i

Per chunk (7 pairs = 14 out rows) per mt (128 out ch): 4 PSUM tiles m_i
[128,7,56], each accumulating 6 matmuls (2 kt x 3 dx) of FD=392 -> 24
matmuls x ~398cyc @2.4GHz = 4.0us; vs direct 36 x 454. Per-core PE stream:
768 matmuls = ~127us (direct: ~190us).

The vertical transform runs on the DVE as stride-1 tensor_tensor fp16 ops
(2x mode; the stride-2 row pairing lives in an outer AP dim, which is free),
~2.2us per chunk. The output transform drains PSUM via 4 ScalarE copies
(fp32->fp16) then 4 DVE ops; bias is fused into the final scalar_tensor_tensor
adds. All of DVE (~4.3us) / ScalarE (~3.8us) stay under the PE's 8us per
chunk, so the kernel remains PE-bound.

Schedule notes (NTFF-trace driven; measured ~161us max-core vs 223us for
the direct implicit-GEMM kernel, rel err 8e-4):
- Span = framework preamble (7.4us, up to 13.4us on cores with engine-start
  skew) + head DMA wait + ~131us PE stream + drain tail + ~8us framework
  teardown (per-engine semaphore resets).  The stream itself profiles
  stall-free at the 768-matmul floor on a quiet core.
- 112 junk warmup matmuls flip the HAM clock gate 4/8 -> 8/8 and bridge the
  whole head-DMA wait even on a slow core (junk ending early lets the PE
  idle, which re-throttles the gate: ~1.8us of half-rate stream).
- All 8 cores saturate HBM at startup, so head DMAs are ordered: chunk-0
  x rows, then mt0 weights in half-slices, then mt1 weights, then bulk x;
  spread across sync/scalar HWDGE + gpsimd SWDGE.  Chunk 0 runs kt-major so
  kt1 operands are needed ~12 matmuls in.
- Output DMA issues ride the sync queue only: a dma_start on the scalar
  queue costs ~630ns between ACTIVATEs and delays PSUM recycling.
- Look-ahead: each chunk's DVE transforms are issued one chunk early
  (before the current chunk's drains on the DVE queue); image n+1's DMA is
  issued at (n, c0) so those transforms follow it in program order -- Tile
  dependency tracking is program-order based.
- fp16 x/out DMA (host converts); final chunk-mt split 4+3 pairs so the
  last drain+DMA overlaps trailing matmuls.
"""

import numpy as np

# Problem constants (hardcoded per contract; kernel.py must be self-contained)
OOC, OIC, K1, K2 = 64, 64, 3, 3
R0, R1 = 4, 4
N_CORES = 8
BATCH = 32
N_PER_CORE = BATCH // N_CORES  # 4
C = 256
H = W = 56
HP = WP = H + 2  # zero-padded spatial (padding applied on host)
KT = C // 128    # 2 input-channel tiles
MT = C // 128    # 2 output-channel tiles
YP = 4           # Winograd F(2,3) positions
DX = K2          # 3 horizontal taps
PPC = 7          # row pairs per chunk
NCH = (H // 2) // PPC  # 4 chunks per image

_NC_CACHE = {}
LAST_RESULT = {}  # test.py introspection: last BassKernelResults


def _expand_weight(weight, alphas, betas):
    """W[p0*64+i, p1*64+j, ky, kx] = w[i,j,ky,kx] * a[p0,p1] / (1+exp(w*b[p0,p1]))."""
    w = weight.astype(np.float32)[None, None]            # (1,1,64,64,3,3)
    a = alphas.astype(np.float32).reshape(R0, R1)[:, :, None, None, None, None]
    b = betas.astype(np.float32).reshape(R0, R1)[:, :, None, None, None, None]
    act = w * a / (1.0 + np.exp(w * b))                  # (4,4,64,64,3,3)
    return act.transpose(0, 2, 1, 3, 4, 5).reshape(R0 * OOC, R1 * OIC, K1, K2)


def _host_prep(x, weight, alphas, betas, bias):
    x = np.asarray(x, dtype=np.float32).astype(np.float16)
    xpad = np.pad(x, ((0, 0), (0, 0), (1, 1), (1, 1)))
    Wfull = _expand_weight(np.asarray(weight), np.asarray(alphas),
                           np.asarray(betas))            # (co256,ci256,3,3)
    # Vertical Winograd weight transform G @ g over ky:
    # u0=g0, u1=(g0+g1+g2)/2, u2=(g0-g1+g2)/2, u3=g2
    g0, g1, g2 = Wfull[:, :, 0, :], Wfull[:, :, 1, :], Wfull[:, :, 2, :]
    Wy = np.stack([g0, (g0 + g1 + g2) * 0.5, (g0 - g1 + g2) * 0.5, g2])
    # Wy: (yp, co, ci, dx) -> w_arr[ci_l, kt, mt, yp, dx, co_l]
    t = Wy.reshape(YP, MT, 128, KT, 128, DX)             # (yp,mt,co_l,kt,ci_l,dx)
    w_arr = np.ascontiguousarray(
        t.transpose(4, 3, 1, 0, 5, 2)).astype(np.float16)
    b2 = np.asarray(bias, dtype=np.float32).reshape(MT, 128).T  # [128, MT]
    b_arr = np.ascontiguousarray(np.stack([b2, -b2], axis=-1))  # [128, MT, 2]
    return xpad, w_arr, b_arr


def _build_nc():
    import concourse.mybir as mybir
    import concourse.tile as tile
    from concourse import bacc

    fp32 = mybir.dt.float32
    fp16 = mybir.dt.float16

    nc = bacc.Bacc("TRN2", target_bir_lowering=False, debug=False,
                   num_devices=N_CORES)

    x_d = nc.dram_tensor("x", [N_PER_CORE, C, HP, WP], fp16,
                         kind="ExternalInput")
    w_d = nc.dram_tensor("w", [128, KT, MT, YP, DX, 128], fp16,
                         kind="ExternalInput")
    b_d = nc.dram_tensor("b", [128, MT, 2], fp32, kind="ExternalInput")
    o_d = nc.dram_tensor("out", [N_PER_CORE, C, H, W], fp16,
                         kind="ExternalOutput")

    # Two HWDGE rings: sync carries kt=0 input traffic + mt=0 outputs,
    # scalar carries kt=1 inputs + mt=1 outputs.
    def ring(kt):
        return nc.sync if kt == 0 else nc.scalar

    with tile.TileContext(nc) as tc:
        with (
            tc.tile_pool(name="const", bufs=1) as const_pool,
            tc.tile_pool(name="xpad", bufs=1) as xp_pool,
            tc.tile_pool(name="tt", bufs=3) as tt_pool,
            tc.tile_pool(name="mm", bufs=3) as mm_pool,
            tc.tile_pool(name="aa", bufs=3) as aa_pool,
            tc.tile_pool(name="ot", bufs=4) as out_pool,
            tc.tile_pool(name="ps", bufs=7, space="PSUM") as psum_pool,
        ):
            w_sb = const_pool.tile([128, KT, MT, YP, DX, 128], fp16,
                                   name="w_sb", tag="w_sb")
            b_sb = const_pool.tile([128, MT, 2], fp32, name="b_sb",
                                   tag="b_sb")

            # PE warmup: junk matmuls on scratch SBUF during the initial DMA
            # wait flip the HAM clock gate to 8/8 before the real stream.
            warm_in = const_pool.tile([128, 128], fp16, name="warm_in",
                                      tag="warm_in")
            warm_ps = psum_pool.tile([128, 64], fp32, name="warm_ps",
                                     tag="warm_ps", bufs=1)
            # 112: long enough that on a slow-preamble core the junk bridges
            # the whole head-DMA wait (junk ending early lets the PE idle and
            # re-throttles the HAM gate, ~1.8us of half-rate stream)
            nc.gpsimd.memset(warm_in[:], 0.0)
            for _ in range(112):
                nc.tensor.matmul(warm_ps[:], warm_in[:], warm_in[:, 0:64])

            # Double-buffered padded input images (pad arrives from host).
            xp = [[xp_pool.tile([128, HP, WP], fp16, name=f"xp{par}_{kt}",
                                tag=f"xp{par}_{kt}")
                   for kt in range(KT)] for par in range(2)]

            xap = x_d.ap()
            oap = o_d.ap()
            wap = w_d.ap()

            def xdma(eng, n, par, kt, r0, r1):
                eng.dma_start(xp[par][kt][:, r0:r1, :],
                              xap[n, kt * 128:(kt + 1) * 128, r0:r1, :])

            # Head: spread the first chunk's operands across three queues so
            # completions don't serialize, and land ALL FOUR weight slices
            # before the bulk x rows -- on a slow-preamble core chunk (0,0)
            # mt1 otherwise catches up with the w[kt1,mt1] transfer (observed
            # 3.7us PE stall).  Rows 16:30 (chunk-1 transforms) jump ahead of
            # rows 30:58.
            # mt0 weights in half-slices (i0:2 / i2:4) so the first PE
            # blocks' weights land early even when all 8 cores saturate HBM
            # at startup (observed: whole-slice w landing 12us late on a
            # congested core)
            xdma(nc.sync, 0, 0, 0, 0, 16)                      # chunk0 kt0
            xdma(nc.gpsimd, 0, 0, 1, 0, 16)                    # chunk0 kt1
            nc.scalar.dma_start(w_sb[:, 0, 0, 0:2], wap[:, 0, 0, 0:2])
            nc.sync.dma_start(w_sb[:, 1, 0, 0:2], wap[:, 1, 0, 0:2])
            nc.scalar.dma_start(w_sb[:, 0, 0, 2:4], wap[:, 0, 0, 2:4])
            nc.sync.dma_start(w_sb[:, 1, 0, 2:4], wap[:, 1, 0, 2:4])
            nc.scalar.dma_start(w_sb[:, 0, 1], wap[:, 0, 1])   # w kt0 mt1
            nc.sync.dma_start(w_sb[:, 1, 1], wap[:, 1, 1])     # w kt1 mt1
            nc.gpsimd.dma_start(b_sb[:], b_d.ap())             # bias
            xdma(nc.gpsimd, 0, 0, 0, 16, 30)                   # chunk1 kt0
            xdma(nc.scalar, 0, 0, 1, 16, 30)                   # chunk1 kt1
            xdma(nc.sync, 0, 0, 0, 30, 58)                     # rest kt0
            xdma(nc.scalar, 0, 0, 1, 30, 58)                   # rest kt1

            def transforms(par, c, kt_major=False):
                """DVE vertical Winograd transform for chunk c: 8 stride-1
                fp16 tensor_tensor ops; returns tt tile [128,KT,YP,PPC,WP]."""
                tt = tt_pool.tile([128, KT, YP, PPC, WP], fp16,
                                  name="tt", tag="tt")
                r = 14 * c
                order = ([(kt, i) for kt in range(KT) for i in range(YP)]
                         if kt_major else
                         [(kt, i) for i in range(YP) for kt in range(KT)])
                for kt, i in order:
                    xk = xp[par][kt]
                    # row APs: stride-2 over pairs lives in an outer AP dim
                    e0 = xk[:, r + 0:r + 13:2, :]   # x[2p]
                    e1 = xk[:, r + 1:r + 14:2, :]   # x[2p+1]
                    e2 = xk[:, r + 2:r + 15:2, :]   # x[2p+2]
                    e3 = xk[:, r + 3:r + 16:2, :]   # x[2p+3]
                    dst = tt[:, kt, i]
                    if i == 0:
                        nc.vector.tensor_sub(dst, e0, e2)
                    elif i == 1:
                        nc.vector.tensor_add(dst, e1, e2)
                    elif i == 2:
                        nc.vector.tensor_sub(dst, e2, e1)
                    else:
                        nc.vector.tensor_sub(dst, e1, e3)
                return tt

            def chunk_mms(tt, mt, kt_major=False, cushion=0, rows=PPC,
                          p0=0):
                """PE: 4 PSUM tiles m_i, each 6 matmuls (kt x dx), FD=56*rows."""
                ps = [psum_pool.tile([128, rows, W], fp32, name="ps",
                                     tag="ps") for _ in range(YP)]
                if kt_major:
                    seq = [(kt, i, dx) for kt in range(KT) for i in range(YP)
                           for dx in range(DX)]
                else:
                    seq = [(kt, i, dx) for i in range(YP) for kt in range(KT)
                           for dx in range(DX)]
                nblk = 0
                for kt, i, dx in seq:
                    nc.tensor.matmul(
                        ps[i][:, :, :],
                        w_sb[:, kt, mt, i, dx, :],
                        tt[:, kt, i, p0:p0 + rows, dx:dx + W],
                        start=(kt == 0 and dx == 0),
                        stop=(kt == KT - 1 and dx == DX - 1),
                    )
                    nblk += 1
                    if cushion and nblk % DX == 0 and nblk < len(seq):
                        for _ in range(cushion):
                            nc.tensor.matmul(warm_ps[:], warm_in[:],
                                             warm_in[:, 0:64])
                return ps

            def drain(n, mt, c, ps, rows=PPC, p0=0):
                """ScalarE: PSUM->SBUF fp16 copies; DVE: output transform with
                fused bias; DMA 14 contiguous output rows."""
                mm = mm_pool.tile([128, YP, rows, W], fp16, name="mm",
                                  tag="mm")
                # bias folded into the PSUM drains: mm0 = m0+b, mm3 = m3-b,
                # so out0 = mm0+m1+m2 and out1 = m1-m2-mm3 need only plain
                # tensor_tensor ops on the DVE (scalar_tensor_tensor runs at
                # 1x mode, ~612ns vs ~359ns measured).  Identity for all four
                # so the act-function table never swaps.
                Ident = mybir.ActivationFunctionType.Identity
                nc.scalar.activation(mm[:, 0], ps[0][:], Ident,
                                     bias=b_sb[:, mt, 0:1])
                nc.scalar.activation(mm[:, 1], ps[1][:], Ident)
                nc.scalar.activation(mm[:, 2], ps[2][:], Ident)
                nc.scalar.activation(mm[:, 3], ps[3][:], Ident,
                                     bias=b_sb[:, mt, 1:2])
                a0 = aa_pool.tile([128, rows, W], fp16, name="a0", tag="aa")
                a1 = aa_pool.tile([128, rows, W], fp16, name="a1", tag="aa")
                ot = out_pool.tile([128, rows, 2, W], fp16, name="ot",
                                   tag="ot")
                nc.vector.tensor_add(a0, mm[:, 0], mm[:, 1])
                nc.vector.tensor_add(ot[:, :, 0, :], a0[:], mm[:, 2])
                nc.vector.tensor_sub(a1, mm[:, 1], mm[:, 2])
                nc.vector.tensor_sub(ot[:, :, 1, :], a1[:], mm[:, 3])
                y0 = 14 * c + 2 * p0
                # all output issues ride the (mostly idle) sync queue: a
                # dma_start on the scalar queue costs ~630ns of queue time
                # between ACTIVATEs and delays PSUM recycling (observed
                # 1.4us PE stall on slow cores)
                nc.sync.dma_start(
                    oap[n, mt * 128:(mt + 1) * 128, y0:y0 + 2 * rows, :],
                    ot[:])

            # ---- chunk (0,0): special head scheduling ----
            # image 1's whole-image DMA queues behind the head operands on
            # the rings; it must be issued before any transform reads xp[1].
            for kt in range(KT):
                xdma(ring(kt), 1, 1, kt, 0, 58)
            tt0 = transforms(0, 0, kt_major=True)
            tt_next = transforms(0, 1)
            ps = chunk_mms(tt0, 0, kt_major=True, cushion=1)
            drain(0, 0, 0, ps)
            ps = chunk_mms(tt0, 1, kt_major=True)
            drain(0, 1, 0, ps)

            # ---- steady state ----
            for n in range(N_PER_CORE):
                par = n % 2
                if n > 0 and n + 1 < N_PER_CORE:
                    # image n+1's DMA, issued one image ahead of use so the
                    # look-ahead transforms for (n+1, 0) see it in program
                    # order (xp[(n+1) % 2]'s prior readers are all issued);
                    # kt1 rides gpsimd SWDGE to keep the scalar queue pure
                    # ACTIVATEs
                    xdma(nc.sync, n + 1, (n + 1) % 2, 0, 0, 58)
                    xdma(nc.gpsimd, n + 1, (n + 1) % 2, 1, 0, 58)
                for c in range(1 if n == 0 else 0, NCH):
                    tt = tt_next
                    # issue next chunk's transforms ahead of this chunk's
                    # drains on the DVE queue (they only depend on x DMA)
                    nc_, cc = (n, c + 1) if c + 1 < NCH else (n + 1, 0)
                    if nc_ < N_PER_CORE:
                        tt_next = transforms(nc_ % 2, cc)
                    last = (n == N_PER_CORE - 1 and c == NCH - 1)
                    for mt in range(MT):
                        if last and mt == MT - 1:
                            # split the final chunk-mt so the last drain+DMA
                            # overlaps trailing matmuls
                            ps = chunk_mms(tt, mt, rows=4, p0=0)
                            drain(n, mt, c, ps, rows=4, p0=0)
                            ps = chunk_mms(tt, mt, rows=3, p0=4)
                            drain(n, mt, c, ps, rows=3, p0=4)
                        else:
                            ps = chunk_mms(tt, mt)
                            drain(n, mt, c, ps)
    nc.compile()
    return nc


def get_nc():
    if "nc" not in _NC_CACHE:
        _NC_CACHE["nc"] = _build_nc()
    return _NC_CACHE["nc"]


def _in_maps(xpad, w_arr, b_arr):
    return [
        {"x": xpad[i * N_PER_CORE:(i + 1) * N_PER_CORE], "w": w_arr,
         "b": b_arr}
        for i in range(N_CORES)
    ]


def kernel(x, weight, alphas, betas, bias):
    from concourse.bass_utils import run_bass_kernel_spmd

    xpad, w_arr, b_arr = _host_prep(x, weight, alphas, betas, bias)
    nc = get_nc()
    in_maps = _in_maps(xpad, w_arr, b_arr)
    res = run_bass_kernel_spmd(nc, in_maps, core_ids=list(range(N_CORES)))
    LAST_RESULT["res"] = res
    return np.concatenate([r["out"] for r in res.results],
                          axis=0).astype(np.float32)


# revision 26
# speedup vs baseline: 1.0013x; 1.0013x over previous
"""Trainium2 Bass kernel: 3x3 conv (NCHW 32x256x56x56, 256->256ch, pad 1) with
a host-expanded synthesized weight, data-parallel over 8 NeuronCores.

Vertical 1D Winograd F(2,3): for each output row pair (y=2p, 2p+1) the three
vertical taps collapse into 4 Winograd products, cutting PE work 1.5x vs
direct implicit GEMM (12 row-matmuls per 2 rows -> 8).

  t0 = x[2p] - x[2p+2]      u0 = g0            out[2p]   = m0+m1+m2 + b
  t1 = x[2p+1] + x[2p+2]    u1 = (g0+g1+g2)/2  out[2p+1] = m1-m2-m3 + b
  t2 = x[2p+2] - x[2p+1]    u2 = (g0-g1+g2)/2
  t3 = x[2p+1] - x[2p+3]    u3 = g2            m_i = sum_{ci,dx} t_i * u_i

Per chunk (7 pairs = 14 out rows) per mt (128 out ch): 4 PSUM tiles m_i
[128,7,56], each accumulating 6 matmuls (2 kt x 3 dx) of FD=392 -> 24
matmuls x ~398cyc @2.4GHz = 4.0us; vs direct 36 x 454. Per-core PE stream:
768 matmuls = ~127us (direct: ~190us).

The vertical transform runs on the DVE as stride-1 tensor_tensor fp16 ops
(2x mode; the stride-2 row pairing lives in an outer AP dim, which is free),
~2.2us per chunk. The output transform drains PSUM via 4 ScalarE copies
(fp32->fp16) then 4 DVE ops; bias is fused into the final scalar_tensor_tensor
adds. All of DVE (~4.3us) / ScalarE (~3.8us) stay under the PE's 8us per
chunk, so the kernel remains PE-bound.

Schedule notes (NTFF-trace driven; measured ~161us max-core vs 223us for
the direct implicit-GEMM kernel, rel err 8e-4):
- Span = framework preamble (7.4us, up to 13.4us on cores with engine-start
  skew) + head DMA wait + ~131us PE stream + drain tail + ~8us framework
  teardown (per-engine semaphore resets).  The stream itself profiles
  stall-free at the 768-matmul floor on a quiet core.
- 112 junk warmup matmuls flip the HAM clock gate 4/8 -> 8/8 and bridge the
  whole head-DMA wait even on a slow core (junk ending early lets the PE
  idle, which re-throttles the gate: ~1.8us of half-rate stream).
- All 8 cores saturate HBM at startup, so head DMAs are ordered: chunk-0
  x rows, then mt0 weights in half-slices, then mt1 weights, then bulk x;
  spread across sync/scalar HWDGE + gpsimd SWDGE.  Chunk 0 runs kt-major so
  kt1 operands are needed ~12 matmuls in.
- Output DMA issues ride the sync queue only: a dma_start on the scalar
  queue costs ~630ns between ACTIVATEs and delays PSUM recycling.
- Look-ahead: each chunk's DVE transforms are issued one chunk early
  (before the current chunk's drains on the DVE queue); image n+1's DMA is
  issued at (n, c0) so those transforms follow it in program order -- Tile
  dependency tracking is program-order based.
- fp16 x/out DMA (host converts); final chunk-mt split 4+3 pairs so the
  last drain+DMA overlaps trailing matmuls.
"""

import numpy as np

# Problem constants (hardcoded per contract; kernel.py must be self-contained)
OOC, OIC, K1, K2 = 64, 64, 3, 3
R0, R1 = 4, 4
N_CORES = 8
BATCH = 32
N_PER_CORE = BATCH // N_CORES  # 4
C = 256
H = W = 56
HP = WP = H + 2  # zero-padded spatial (padding applied on host)
KT = C // 128    # 2 input-channel tiles
MT = C // 128    # 2 output-channel tiles
YP = 4           # Winograd F(2,3) positions
DX = K2          # 3 horizontal taps
PPC = 7          # row pairs per chunk
NCH = (H // 2) // PPC  # 4 chunks per image

_NC_CACHE = {}
LAST_RESULT = {}  # test.py introspection: last BassKernelResults


def _expand_weight(weight, alphas, betas):
    """W[p0*64+i, p1*64+j, ky, kx] = w[i,j,ky,kx] * a[p0,p1] / (1+exp(w*b[p0,p1]))."""
    w = weight.astype(np.float32)[None, None]            # (1,1,64,64,3,3)
    a = alphas.astype(np.float32).reshape(R0, R1)[:, :, None, None, None, None]
    b = betas.astype(np.float32).reshape(R0, R1)[:, :, None, None, None, None]
    act = w * a / (1.0 + np.exp(w * b))                  # (4,4,64,64,3,3)
    return act.transpose(0, 2, 1, 3, 4, 5).reshape(R0 * OOC, R1 * OIC, K1, K2)


def _host_prep(x, weight, alphas, betas, bias):
    x = np.asarray(x, dtype=np.float32).astype(np.float16)
    xpad = np.pad(x, ((0, 0), (0, 0), (1, 1), (1, 1)))
    Wfull = _expand_weight(np.asarray(weight), np.asarray(alphas),
                           np.asarray(betas))            # (co256,ci256,3,3)
    # Vertical Winograd weight transform G @ g over ky:
    # u0=g0, u1=(g0+g1+g2)/2, u2=(g0-g1+g2)/2, u3=g2
    g0, g1, g2 = Wfull[:, :, 0, :], Wfull[:, :, 1, :], Wfull[:, :, 2, :]
    Wy = np.stack([g0, (g0 + g1 + g2) * 0.5, (g0 - g1 + g2) * 0.5, g2])
    # Wy: (yp, co, ci, dx) -> w_arr[ci_l, kt, mt, yp, dx, co_l]
    t = Wy.reshape(YP, MT, 128, KT, 128, DX)             # (yp,mt,co_l,kt,ci_l,dx)
    w_arr = np.ascontiguousarray(
        t.transpose(4, 3, 1, 0, 5, 2)).astype(np.float16)
    b2 = np.asarray(bias, dtype=np.float32).reshape(MT, 128).T  # [128, MT]
    b_arr = np.ascontiguousarray(np.stack([b2, -b2], axis=-1))  # [128, MT, 2]
    return xpad, w_arr, b_arr


def _build_nc():
    import concourse.mybir as mybir
    import concourse.tile as tile
    from concourse import bacc

    fp32 = mybir.dt.float32
    fp16 = mybir.dt.float16

    nc = bacc.Bacc("TRN2", target_bir_lowering=False, debug=False,
                   num_devices=N_CORES)

    x_d = nc.dram_tensor("x", [N_PER_CORE, C, HP, WP], fp16,
                         kind="ExternalInput")
    w_d = nc.dram_tensor("w", [128, KT, MT, YP, DX, 128], fp16,
                         kind="ExternalInput")
    b_d = nc.dram_tensor("b", [128, MT, 2], fp32, kind="ExternalInput")
    o_d = nc.dram_tensor("out", [N_PER_CORE, C, H, W], fp16,
                         kind="ExternalOutput")

    # Two HWDGE rings: sync carries kt=0 input traffic + mt=0 outputs,
    # scalar carries kt=1 inputs + mt=1 outputs.
    def ring(kt):
        return nc.sync if kt == 0 else nc.scalar

    with tile.TileContext(nc) as tc:
        with (
            tc.tile_pool(name="const", bufs=1) as const_pool,
            tc.tile_pool(name="xpad", bufs=1) as xp_pool,
            tc.tile_pool(name="tt", bufs=3) as tt_pool,
            tc.tile_pool(name="mm", bufs=3) as mm_pool,
            tc.tile_pool(name="aa", bufs=3) as aa_pool,
            tc.tile_pool(name="ot", bufs=4) as out_pool,
            tc.tile_pool(name="ps", bufs=7, space="PSUM") as psum_pool,
        ):
            w_sb = const_pool.tile([128, KT, MT, YP, DX, 128], fp16,
                                   name="w_sb", tag="w_sb")
            b_sb = const_pool.tile([128, MT, 2], fp32, name="b_sb",
                                   tag="b_sb")

            # PE warmup: junk matmuls on scratch SBUF during the initial DMA
            # wait flip the HAM clock gate to 8/8 before the real stream.
            warm_in = const_pool.tile([128, 128], fp16, name="warm_in",
                                      tag="warm_in")
            warm_ps = psum_pool.tile([128, 64], fp32, name="warm_ps",
                                     tag="warm_ps", bufs=1)
            # 112: long enough that on a slow-preamble core the junk bridges
            # the whole head-DMA wait (junk ending early lets the PE idle and
            # re-throttles the HAM gate, ~1.8us of half-rate stream)
            nc.gpsimd.memset(warm_in[:], 0.0)
            for _ in range(112):
                nc.tensor.matmul(warm_ps[:], warm_in[:], warm_in[:, 0:64])

            # Double-buffered padded input images (pad arrives from host).
            xp = [[xp_pool.tile([128, HP, WP], fp16, name=f"xp{par}_{kt}",
                                tag=f"xp{par}_{kt}")
                   for kt in range(KT)] for par in range(2)]

            xap = x_d.ap()
            oap = o_d.ap()
            wap = w_d.ap()

            def xdma(eng, n, par, kt, r0, r1):
                eng.dma_start(xp[par][kt][:, r0:r1, :],
                              xap[n, kt * 128:(kt + 1) * 128, r0:r1, :])

            # Head: spread the first chunk's operands across three queues so
            # completions don't serialize, and land ALL FOUR weight slices
            # before the bulk x rows -- on a slow-preamble core chunk (0,0)
            # mt1 otherwise catches up with the w[kt1,mt1] transfer (observed
            # 3.7us PE stall).  Rows 16:30 (chunk-1 transforms) jump ahead of
            # rows 30:58.
            # mt0 weights in half-slices (i0:2 / i2:4) so the first PE
            # blocks' weights land early even when all 8 cores saturate HBM
            # at startup (observed: whole-slice w landing 12us late on a
            # congested core)
            xdma(nc.sync, 0, 0, 0, 0, 16)                      # chunk0 kt0
            xdma(nc.gpsimd, 0, 0, 1, 0, 16)                    # chunk0 kt1
            nc.scalar.dma_start(w_sb[:, 0, 0, 0:2], wap[:, 0, 0, 0:2])
            nc.sync.dma_start(w_sb[:, 1, 0, 0:2], wap[:, 1, 0, 0:2])
            nc.scalar.dma_start(w_sb[:, 0, 0, 2:4], wap[:, 0, 0, 2:4])
            nc.sync.dma_start(w_sb[:, 1, 0, 2:4], wap[:, 1, 0, 2:4])
            nc.scalar.dma_start(w_sb[:, 0, 1], wap[:, 0, 1])   # w kt0 mt1
            nc.sync.dma_start(w_sb[:, 1, 1], wap[:, 1, 1])     # w kt1 mt1
            nc.gpsimd.dma_start(b_sb[:], b_d.ap())             # bias
            xdma(nc.gpsimd, 0, 0, 0, 16, 30)                   # chunk1 kt0
            xdma(nc.scalar, 0, 0, 1, 16, 30)                   # chunk1 kt1
            xdma(nc.sync, 0, 0, 0, 30, 58)                     # rest kt0
            xdma(nc.scalar, 0, 0, 1, 30, 58)                   # rest kt1

            def transforms(par, c, kt_major=False):
                """DVE vertical Winograd transform for chunk c: 8 stride-1
                fp16 tensor_tensor ops; returns tt tile [128,KT,YP,PPC,WP]."""
                tt = tt_pool.tile([128, KT, YP, PPC, WP], fp16,
                                  name="tt", tag="tt")
                r = 14 * c
                order = ([(kt, i) for kt in range(KT) for i in range(YP)]
                         if kt_major else
                         [(kt, i) for i in range(YP) for kt in range(KT)])
                for kt, i in order:
                    xk = xp[par][kt]
                    # row APs: stride-2 over pairs lives in an outer AP dim
                    e0 = xk[:, r + 0:r + 13:2, :]   # x[2p]
                    e1 = xk[:, r + 1:r + 14:2, :]   # x[2p+1]
                    e2 = xk[:, r + 2:r + 15:2, :]   # x[2p+2]
                    e3 = xk[:, r + 3:r + 16:2, :]   # x[2p+3]
                    dst = tt[:, kt, i]
                    if i == 0:
                        nc.vector.tensor_sub(dst, e0, e2)
                    elif i == 1:
                        nc.vector.tensor_add(dst, e1, e2)
                    elif i == 2:
                        nc.vector.tensor_sub(dst, e2, e1)
                    else:
                        nc.vector.tensor_sub(dst, e1, e3)
                return tt

            def chunk_mms(tt, mt, kt_major=False, cushion=0, rows=PPC,
                          p0=0):
                """PE: 4 PSUM tiles m_i, each 6 matmuls (kt x dx), FD=56*rows."""
                ps = [psum_pool.tile([128, rows, W], fp32, name="ps",
                                     tag="ps") for _ in range(YP)]
                if kt_major:
                    seq = [(kt, i, dx) for kt in range(KT) for i in range(YP)
                           for dx in range(DX)]
                else:
                    seq = [(kt, i, dx) for i in range(YP) for kt in range(KT)
                           for dx in range(DX)]
                nblk = 0
                for kt, i, dx in seq:
                    nc.tensor.matmul(
                        ps[i][:, :, :],
                        w_sb[:, kt, mt, i, dx, :],
                        tt[:, kt, i, p0:p0 + rows, dx:dx + W],
                        start=(kt == 0 and dx == 0),
                        stop=(kt == KT - 1 and dx == DX - 1),
                    )
                    nblk += 1
                    if cushion and nblk % DX == 0 and nblk < len(seq):
                        for _ in range(cushion):
                            nc.tensor.matmul(warm_ps[:], warm_in[:],
                                             warm_in[:, 0:64])
                return ps

            def drain(n, mt, c, ps, rows=PPC, p0=0):
                """ScalarE: PSUM->SBUF fp16 copies; DVE: output transform with
                fused bias; DMA 14 contiguous output rows."""
                mm = mm_pool.tile([128, YP, rows, W], fp16, name="mm",
                                  tag="mm")
                # bias folded into the PSUM drains: mm0 = m0+b, mm3 = m3-b,
                # so out0 = mm0+m1+m2 and out1 = m1-m2-mm3 need only plain
                # tensor_tensor ops on the DVE (scalar_tensor_tensor runs at
                # 1x mode, ~612ns vs ~359ns measured).  Identity for all four
                # so the act-function table never swaps.
                Ident = mybir.ActivationFunctionType.Identity
                nc.scalar.activation(mm[:, 0], ps[0][:], Ident,
                                     bias=b_sb[:, mt, 0:1])
                nc.scalar.activation(mm[:, 1], ps[1][:], Ident)
                nc.scalar.activation(mm[:, 2], ps[2][:], Ident)
                nc.scalar.activation(mm[:, 3], ps[3][:], Ident,
                                     bias=b_sb[:, mt, 1:2])
                a0 = aa_pool.tile([128, rows, W], fp16, name="a0", tag="aa")
                a1 = aa_pool.tile([128, rows, W], fp16, name="a1", tag="aa")
                ot = out_pool.tile([128, rows, 2, W], fp16, name="ot",
                                   tag="ot")
                nc.vector.tensor_add(a0, mm[:, 0], mm[:, 1])
                nc.vector.tensor_add(ot[:, :, 0, :], a0[:], mm[:, 2])
                nc.vector.tensor_sub(a1, mm[:, 1], mm[:, 2])
                nc.vector.tensor_sub(ot[:, :, 1, :], a1[:], mm[:, 3])
                y0 = 14 * c + 2 * p0
                # all output issues ride the (mostly idle) sync queue: a
                # dma_start on the scalar queue costs ~630ns of queue time
                # between ACTIVATEs and delays PSUM recycling (observed
                # 1.4us PE stall on slow cores)
                nc.sync.dma_start(
                    oap[n, mt * 128:(mt + 1) * 128, y0:y0 + 2 * rows, :],
                    ot[:])

            # ---- chunk (0,0): special head scheduling ----
            # image 1's whole-image DMA queues behind the head operands on
            # the rings; it must be issued before any transform reads xp[1].
            for kt in range(KT):
                xdma(ring(kt), 1, 1, kt, 0, 58)
            tt0 = transforms(0, 0, kt_major=True)
            tt_next = transforms(0, 1)
            ps = chunk_mms(tt0, 0, kt_major=True, cushion=1)
            drain(0, 0, 0, ps)
            ps = chunk_mms(tt0, 1, kt_major=True)
            drain(0, 1, 0, ps)

            # ---- steady state ----
            for n in range(N_PER_CORE):
                par = n % 2
                if n > 0 and n + 1 < N_PER_CORE:
                    # image n+1's DMA, issued one image ahead of use so the
                    # look-ahead transforms for (n+1, 0) see it in program
                    # order (xp[(n+1) % 2]'s prior readers are all issued);
                    # kt1 rides gpsimd SWDGE to keep the scalar queue pure
                    # ACTIVATEs
                    xdma(nc.sync, n + 1, (n + 1) % 2, 0, 0, 58)
                    xdma(nc.gpsimd, n + 1, (n + 1) % 2, 1, 0, 58)
                for c in range(1 if n == 0 else 0, NCH):
                    tt = tt_next
                    # issue next chunk's transforms ahead of this chunk's
                    # drains on the DVE queue (they only depend on x DMA)
                    nc_, cc = (n, c + 1) if c + 1 < NCH else (n + 1, 0)
                    if nc_ < N_PER_CORE:
                        tt_next = transforms(nc_ % 2, cc)
                    last = (n == N_PER_CORE - 1 and c == NCH - 1)
                    for mt in range(MT):
                        if last and mt == MT - 1:
                            # split the final chunk-mt 4+2+1 so the last
                            # drain chain (4 serialized ACTIVATEs + DVE +
                            # DMA) runs on a single row pair and the rest
                            # overlaps trailing matmuls
                            for rows, p0 in ((4, 0), (2, 4), (1, 6)):
                                ps = chunk_mms(tt, mt, rows=rows, p0=p0)
                                drain(n, mt, c, ps, rows=rows, p0=p0)
                        else:
                            ps = chunk_mms(tt, mt)
                            drain(n, mt, c, ps)
    nc.compile()
    return nc


def get_nc():
    if "nc" not in _NC_CACHE:
        _NC_CACHE["nc"] = _build_nc()
    return _NC_CACHE["nc"]


def _in_maps(xpad, w_arr, b_arr):
    return [
        {"x": xpad[i * N_PER_CORE:(i + 1) * N_PER_CORE], "w": w_arr,
         "b": b_arr}
        for i in range(N_CORES)
    ]


def kernel(x, weight, alphas, betas, bias):
    from concourse.bass_utils import run_bass_kernel_spmd

    xpad, w_arr, b_arr = _host_prep(x, weight, alphas, betas, bias)
    nc = get_nc()
    in_maps = _in_maps(xpad, w_arr, b_arr)
    res = run_bass_kernel_spmd(nc, in_maps, core_ids=list(range(N_CORES)))
    LAST_RESULT["res"] = res
    return np.concatenate([r["out"] for r in res.results],
                          axis=0).astype(np.float32)


# revision 27
# speedup vs baseline: 1.0052x; 1.0039x over previous
"""Trainium2 Bass kernel: 3x3 conv (NCHW 32x256x56x56, 256->256ch, pad 1) with
a host-expanded synthesized weight, data-parallel over 8 NeuronCores.

Vertical 1D Winograd F(2,3): for each output row pair (y=2p, 2p+1) the three
vertical taps collapse into 4 Winograd products, cutting PE work 1.5x vs
direct implicit GEMM (12 row-matmuls per 2 rows -> 8).

  t0 = x[2p] - x[2p+2]      u0 = g0            out[2p]   = m0+m1+m2 + b
  t1 = x[2p+1] + x[2p+2]    u1 = (g0+g1+g2)/2  out[2p+1] = m1-m2-m3 + b
  t2 = x[2p+2] - x[2p+1]    u2 = (g0-g1+g2)/2
  t3 = x[2p+1] - x[2p+3]    u3 = g2            m_i = sum_{ci,dx} t_i * u_i

Per chunk (7 pairs = 14 out rows) per mt (128 out ch): 4 PSUM tiles m_i
[128,7,56], each accumulating 6 matmuls (2 kt x 3 dx) of FD=392 -> 24
matmuls x ~398cyc @2.4GHz = 4.0us; vs direct 36 x 454. Per-core PE stream:
768 matmuls = ~127us (direct: ~190us).

The vertical transform runs on the DVE as stride-1 tensor_tensor fp16 ops
(2x mode; the stride-2 row pairing lives in an outer AP dim, which is free),
~2.2us per chunk. The output transform drains PSUM via 4 ScalarE copies
(fp32->fp16) then 4 DVE ops; bias is fused into the final scalar_tensor_tensor
adds. All of DVE (~4.3us) / ScalarE (~3.8us) stay under the PE's 8us per
chunk, so the kernel remains PE-bound.

Schedule notes (NTFF-trace driven; measured ~161us max-core vs 223us for
the direct implicit-GEMM kernel, rel err 8e-4):
- Span = framework preamble (7.4us, up to 13.4us on cores with engine-start
  skew) + head DMA wait + ~131us PE stream + drain tail + ~8us framework
  teardown (per-engine semaphore resets).  The stream itself profiles
  stall-free at the 768-matmul floor on a quiet core.
- 112 junk warmup matmuls flip the HAM clock gate 4/8 -> 8/8 and bridge the
  whole head-DMA wait even on a slow core (junk ending early lets the PE
  idle, which re-throttles the gate: ~1.8us of half-rate stream).
- All 8 cores saturate HBM at startup, so head DMAs are ordered: chunk-0
  x rows, then mt0 weights in half-slices, then mt1 weights, then bulk x;
  spread across sync/scalar HWDGE + gpsimd SWDGE.  Chunk 0 runs kt-major so
  kt1 operands are needed ~12 matmuls in.
- Output DMA issues ride the sync queue only: a dma_start on the scalar
  queue costs ~630ns between ACTIVATEs and delays PSUM recycling.
- Look-ahead: each chunk's DVE transforms are issued one chunk early
  (before the current chunk's drains on the DVE queue); image n+1's DMA is
  issued at (n, c0) so those transforms follow it in program order -- Tile
  dependency tracking is program-order based.
- fp16 x/out DMA (host converts); final chunk-mt split 4+3 pairs so the
  last drain+DMA overlaps trailing matmuls.
"""

import numpy as np

# Problem constants (hardcoded per contract; kernel.py must be self-contained)
OOC, OIC, K1, K2 = 64, 64, 3, 3
R0, R1 = 4, 4
N_CORES = 8
BATCH = 32
N_PER_CORE = BATCH // N_CORES  # 4
C = 256
H = W = 56
HP = WP = H + 2  # zero-padded spatial (padding applied on host)
KT = C // 128    # 2 input-channel tiles
MT = C // 128    # 2 output-channel tiles
YP = 4           # Winograd F(2,3) positions
DX = K2          # 3 horizontal taps
PPC = 7          # row pairs per chunk
NCH = (H // 2) // PPC  # 4 chunks per image

_NC_CACHE = {}
LAST_RESULT = {}  # test.py introspection: last BassKernelResults


def _expand_weight(weight, alphas, betas):
    """W[p0*64+i, p1*64+j, ky, kx] = w[i,j,ky,kx] * a[p0,p1] / (1+exp(w*b[p0,p1]))."""
    w = weight.astype(np.float32)[None, None]            # (1,1,64,64,3,3)
    a = alphas.astype(np.float32).reshape(R0, R1)[:, :, None, None, None, None]
    b = betas.astype(np.float32).reshape(R0, R1)[:, :, None, None, None, None]
    act = w * a / (1.0 + np.exp(w * b))                  # (4,4,64,64,3,3)
    return act.transpose(0, 2, 1, 3, 4, 5).reshape(R0 * OOC, R1 * OIC, K1, K2)


def _host_prep(x, weight, alphas, betas, bias):
    x = np.asarray(x, dtype=np.float32).astype(np.float16)
    xpad = np.pad(x, ((0, 0), (0, 0), (1, 1), (1, 1)))
    Wfull = _expand_weight(np.asarray(weight), np.asarray(alphas),
                           np.asarray(betas))            # (co256,ci256,3,3)
    # Vertical Winograd weight transform G @ g over ky:
    # u0=g0, u1=(g0+g1+g2)/2, u2=(g0-g1+g2)/2, u3=g2
    g0, g1, g2 = Wfull[:, :, 0, :], Wfull[:, :, 1, :], Wfull[:, :, 2, :]
    Wy = np.stack([g0, (g0 + g1 + g2) * 0.5, (g0 - g1 + g2) * 0.5, g2])
    # Wy: (yp, co, ci, dx) -> w_arr[ci_l, kt, mt, yp, dx, co_l]
    t = Wy.reshape(YP, MT, 128, KT, 128, DX)             # (yp,mt,co_l,kt,ci_l,dx)
    w_arr = np.ascontiguousarray(
        t.transpose(4, 3, 1, 0, 5, 2)).astype(np.float16)
    b2 = np.asarray(bias, dtype=np.float32).reshape(MT, 128).T  # [128, MT]
    b_arr = np.ascontiguousarray(np.stack([b2, -b2], axis=-1))  # [128, MT, 2]
    return xpad, w_arr, b_arr


def _build_nc():
    import concourse.mybir as mybir
    import concourse.tile as tile
    from concourse import bacc

    fp32 = mybir.dt.float32
    fp16 = mybir.dt.float16

    nc = bacc.Bacc("TRN2", target_bir_lowering=False, debug=False,
                   num_devices=N_CORES)

    x_d = nc.dram_tensor("x", [N_PER_CORE, C, HP, WP], fp16,
                         kind="ExternalInput")
    w_d = nc.dram_tensor("w", [128, KT, MT, YP, DX, 128], fp16,
                         kind="ExternalInput")
    b_d = nc.dram_tensor("b", [128, MT, 2], fp32, kind="ExternalInput")
    o_d = nc.dram_tensor("out", [N_PER_CORE, C, H, W], fp16,
                         kind="ExternalOutput")

    # Two HWDGE rings: sync carries kt=0 input traffic + mt=0 outputs,
    # scalar carries kt=1 inputs + mt=1 outputs.
    def ring(kt):
        return nc.sync if kt == 0 else nc.scalar

    with tile.TileContext(nc) as tc:
        with (
            tc.tile_pool(name="const", bufs=1) as const_pool,
            tc.tile_pool(name="xpad", bufs=1) as xp_pool,
            tc.tile_pool(name="tt", bufs=3) as tt_pool,
            tc.tile_pool(name="mm", bufs=3) as mm_pool,
            tc.tile_pool(name="aa", bufs=3) as aa_pool,
            tc.tile_pool(name="ot", bufs=4) as out_pool,
            tc.tile_pool(name="ps", bufs=7, space="PSUM") as psum_pool,
        ):
            w_sb = const_pool.tile([128, KT, MT, YP, DX, 128], fp16,
                                   name="w_sb", tag="w_sb")
            b_sb = const_pool.tile([128, MT, 2], fp32, name="b_sb",
                                   tag="b_sb")

            # PE warmup: junk matmuls on scratch SBUF during the initial DMA
            # wait flip the HAM clock gate to 8/8 before the real stream.
            warm_in = const_pool.tile([128, 128], fp16, name="warm_in",
                                      tag="warm_in")
            warm_ps = psum_pool.tile([128, 64], fp32, name="warm_ps",
                                     tag="warm_ps", bufs=1)
            # 112: long enough that on a slow-preamble core the junk bridges
            # the whole head-DMA wait (junk ending early lets the PE idle and
            # re-throttles the HAM gate, ~1.8us of half-rate stream)
            nc.gpsimd.memset(warm_in[:], 0.0)
            for _ in range(112):
                nc.tensor.matmul(warm_ps[:], warm_in[:], warm_in[:, 0:64])

            # Double-buffered padded input images (pad arrives from host).
            xp = [[xp_pool.tile([128, HP, WP], fp16, name=f"xp{par}_{kt}",
                                tag=f"xp{par}_{kt}")
                   for kt in range(KT)] for par in range(2)]

            xap = x_d.ap()
            oap = o_d.ap()
            wap = w_d.ap()

            def xdma(eng, n, par, kt, r0, r1):
                eng.dma_start(xp[par][kt][:, r0:r1, :],
                              xap[n, kt * 128:(kt + 1) * 128, r0:r1, :])

            # Head: spread the first chunk's operands across three queues so
            # completions don't serialize, and land ALL FOUR weight slices
            # before the bulk x rows -- on a slow-preamble core chunk (0,0)
            # mt1 otherwise catches up with the w[kt1,mt1] transfer (observed
            # 3.7us PE stall).  Rows 16:30 (chunk-1 transforms) jump ahead of
            # rows 30:58.
            # mt0 weights in half-slices (i0:2 / i2:4) so the first PE
            # blocks' weights land early even when all 8 cores saturate HBM
            # at startup (observed: whole-slice w landing 12us late on a
            # congested core)
            xdma(nc.sync, 0, 0, 0, 0, 16)                      # chunk0 kt0
            xdma(nc.gpsimd, 0, 0, 1, 0, 16)                    # chunk0 kt1
            nc.scalar.dma_start(w_sb[:, 0, 0, 0:2], wap[:, 0, 0, 0:2])
            nc.sync.dma_start(w_sb[:, 1, 0, 0:2], wap[:, 1, 0, 0:2])
            nc.scalar.dma_start(w_sb[:, 0, 0, 2:4], wap[:, 0, 0, 2:4])
            nc.sync.dma_start(w_sb[:, 1, 0, 2:4], wap[:, 1, 0, 2:4])
            nc.scalar.dma_start(w_sb[:, 0, 1], wap[:, 0, 1])   # w kt0 mt1
            nc.sync.dma_start(w_sb[:, 1, 1], wap[:, 1, 1])     # w kt1 mt1
            nc.gpsimd.dma_start(b_sb[:], b_d.ap())             # bias
            xdma(nc.gpsimd, 0, 0, 0, 16, 30)                   # chunk1 kt0
            xdma(nc.scalar, 0, 0, 1, 16, 30)                   # chunk1 kt1
            xdma(nc.sync, 0, 0, 0, 30, 58)                     # rest kt0
            xdma(nc.scalar, 0, 0, 1, 30, 58)                   # rest kt1

            def transforms(par, c, kt_major=False):
                """DVE vertical Winograd transform for chunk c: 8 stride-1
                fp16 tensor_tensor ops; returns tt tile [128,KT,YP,PPC,WP]."""
                tt = tt_pool.tile([128, KT, YP, PPC, WP], fp16,
                                  name="tt", tag="tt")
                r = 14 * c
                order = ([(kt, i) for kt in range(KT) for i in range(YP)]
                         if kt_major else
                         [(kt, i) for i in range(YP) for kt in range(KT)])
                for kt, i in order:
                    xk = xp[par][kt]
                    # row APs: stride-2 over pairs lives in an outer AP dim
                    e0 = xk[:, r + 0:r + 13:2, :]   # x[2p]
                    e1 = xk[:, r + 1:r + 14:2, :]   # x[2p+1]
                    e2 = xk[:, r + 2:r + 15:2, :]   # x[2p+2]
                    e3 = xk[:, r + 3:r + 16:2, :]   # x[2p+3]
                    dst = tt[:, kt, i]
                    if i == 0:
                        nc.vector.tensor_sub(dst, e0, e2)
                    elif i == 1:
                        nc.vector.tensor_add(dst, e1, e2)
                    elif i == 2:
                        nc.vector.tensor_sub(dst, e2, e1)
                    else:
                        nc.vector.tensor_sub(dst, e1, e3)
                return tt

            def chunk_mms(tt, mt, kt_major=False, cushion=0, rows=PPC,
                          p0=0):
                """PE: 4 PSUM tiles m_i, each 6 matmuls (kt x dx), FD=56*rows."""
                ps = [psum_pool.tile([128, rows, W], fp32, name="ps",
                                     tag="ps") for _ in range(YP)]
                if kt_major:
                    seq = [(kt, i, dx) for kt in range(KT) for i in range(YP)
                           for dx in range(DX)]
                else:
                    seq = [(kt, i, dx) for i in range(YP) for kt in range(KT)
                           for dx in range(DX)]
                nblk = 0
                for kt, i, dx in seq:
                    nc.tensor.matmul(
                        ps[i][:, :, :],
                        w_sb[:, kt, mt, i, dx, :],
                        tt[:, kt, i, p0:p0 + rows, dx:dx + W],
                        start=(kt == 0 and dx == 0),
                        stop=(kt == KT - 1 and dx == DX - 1),
                    )
                    nblk += 1
                    if cushion and nblk % DX == 0 and nblk < len(seq):
                        for _ in range(cushion):
                            nc.tensor.matmul(warm_ps[:], warm_in[:],
                                             warm_in[:, 0:64])
                return ps

            def drain(n, mt, c, ps, rows=PPC, p0=0):
                """ScalarE: PSUM->SBUF fp16 copies; DVE: output transform with
                fused bias; DMA 14 contiguous output rows."""
                mm = mm_pool.tile([128, YP, rows, W], fp16, name="mm",
                                  tag="mm")
                # bias folded into the PSUM drains: mm0 = m0+b, mm3 = m3-b,
                # so out0 = mm0+m1+m2 and out1 = m1-m2-mm3 need only plain
                # tensor_tensor ops on the DVE (scalar_tensor_tensor runs at
                # 1x mode, ~612ns vs ~359ns measured).  Identity for all four
                # so the act-function table never swaps.
                Ident = mybir.ActivationFunctionType.Identity
                nc.scalar.activation(mm[:, 0], ps[0][:], Ident,
                                     bias=b_sb[:, mt, 0:1])
                nc.scalar.activation(mm[:, 1], ps[1][:], Ident)
                nc.scalar.activation(mm[:, 2], ps[2][:], Ident)
                nc.scalar.activation(mm[:, 3], ps[3][:], Ident,
                                     bias=b_sb[:, mt, 1:2])
                a0 = aa_pool.tile([128, rows, W], fp16, name="a0", tag="aa")
                a1 = aa_pool.tile([128, rows, W], fp16, name="a1", tag="aa")
                ot = out_pool.tile([128, rows, 2, W], fp16, name="ot",
                                   tag="ot")
                nc.vector.tensor_add(a0, mm[:, 0], mm[:, 1])
                nc.vector.tensor_add(ot[:, :, 0, :], a0[:], mm[:, 2])
                nc.vector.tensor_sub(a1, mm[:, 1], mm[:, 2])
                nc.vector.tensor_sub(ot[:, :, 1, :], a1[:], mm[:, 3])
                y0 = 14 * c + 2 * p0
                # all output issues ride the (mostly idle) sync queue: a
                # dma_start on the scalar queue costs ~630ns of queue time
                # between ACTIVATEs and delays PSUM recycling (observed
                # 1.4us PE stall on slow cores)
                nc.sync.dma_start(
                    oap[n, mt * 128:(mt + 1) * 128, y0:y0 + 2 * rows, :],
                    ot[:])

            # ---- chunk (0,0): special head scheduling ----
            # image 1's whole-image DMA queues behind the head operands on
            # the rings; it must be issued before any transform reads xp[1].
            for kt in range(KT):
                xdma(ring(kt), 1, 1, kt, 0, 58)
            tt0 = transforms(0, 0, kt_major=True)
            tt_next = transforms(0, 1)
            ps = chunk_mms(tt0, 0, kt_major=True, cushion=1)
            drain(0, 0, 0, ps)
            ps = chunk_mms(tt0, 1, kt_major=True)
            drain(0, 1, 0, ps)

            # ---- steady state ----
            for n in range(N_PER_CORE):
                par = n % 2
                if n > 0 and n + 1 < N_PER_CORE:
                    # image n+1's DMA, issued one image ahead of use so the
                    # look-ahead transforms for (n+1, 0) see it in program
                    # order (xp[(n+1) % 2]'s prior readers are all issued);
                    # kt1 rides gpsimd SWDGE to keep the scalar queue pure
                    # ACTIVATEs
                    xdma(nc.sync, n + 1, (n + 1) % 2, 0, 0, 58)
                    xdma(nc.gpsimd, n + 1, (n + 1) % 2, 1, 0, 58)
                for c in range(1 if n == 0 else 0, NCH):
                    tt = tt_next
                    # issue next chunk's transforms ahead of this chunk's
                    # drains on the DVE queue (they only depend on x DMA)
                    nc_, cc = (n, c + 1) if c + 1 < NCH else (n + 1, 0)
                    if nc_ < N_PER_CORE:
                        tt_next = transforms(nc_ % 2, cc)
                    last = (n == N_PER_CORE - 1 and c == NCH - 1)
                    for mt in range(MT):
                        if last and mt == MT - 1:
                            # split the final chunk-mt so the last drain+DMA
                            # overlaps trailing matmuls (finer 4+2+1 split
                            # measured slightly worse: extra segment
                            # overhead beats the shorter final chain)
                            ps = chunk_mms(tt, mt, rows=4, p0=0)
                            drain(n, mt, c, ps, rows=4, p0=0)
                            ps = chunk_mms(tt, mt, rows=3, p0=4)
                            drain(n, mt, c, ps, rows=3, p0=4)
                        else:
                            ps = chunk_mms(tt, mt)
                            drain(n, mt, c, ps)
    nc.compile()
    return nc


def get_nc():
    if "nc" not in _NC_CACHE:
        _NC_CACHE["nc"] = _build_nc()
    return _NC_CACHE["nc"]


def _in_maps(xpad, w_arr, b_arr):
    return [
        {"x": xpad[i * N_PER_CORE:(i + 1) * N_PER_CORE], "w": w_arr,
         "b": b_arr}
        for i in range(N_CORES)
    ]


def kernel(x, weight, alphas, betas, bias):
    from concourse.bass_utils import run_bass_kernel_spmd

    xpad, w_arr, b_arr = _host_prep(x, weight, alphas, betas, bias)
    nc = get_nc()
    in_maps = _in_maps(xpad, w_arr, b_arr)
    res = run_bass_kernel_spmd(nc, in_maps, core_ids=list(range(N_CORES)))
    LAST_RESULT["res"] = res
    return np.concatenate([r["out"] for r in res.results],
                          axis=0).astype(np.float32)
